# revision 3
# baseline (speedup 1.0000x reference)
"""Conv1d (B=32, C_in=C_out=64, L=16384, K=3, VALID) on 8 trn2 cores.

Strategy: data-parallel over batch (4 batches/core), polyphase-2 over L.
The host de-interleaves each batch's length axis into even/odd phases
stacked across 128 SBUF partitions: rows 0-63 = x[c, 0::2], rows
64-127 = x[c, 1::2].  The K=3 conv then needs only TWO PSUM-accumulated
matmuls per output chunk (vs 3 for the tap-per-matmul scheme):

  out_even(m) = w0 Xe[m] + w1 Xo[m] + w2 Xe[m+1]
  out_odd(m)  = w0 Xo[m] + w1 Xe[m+1] + w2 Xo[m+1]

  pass A: rhs = [Xe;Xo][:, m],   lhsT_A = [[w0^T, 0   ], [w1^T, w0^T]]
  pass B: rhs = [Xe;Xo][:, m+1], lhsT_B = [[w2^T, w1^T], [0,    w2^T]]

PSUM [128, n] = [out_even ch; out_odd ch]; the host re-interleaves.
This cuts TensorE busy ~41us -> ~28us, taking it off the critical path
(the trace shows PE and DMA alternating as bottleneck at ~70% each).

PSUM->SBUF evacuation (fused bias add, fp32->fp16) alternates whole
512-col chunks between ACT and DVE (one op per bank drain, not two
half-ops, halving per-engine busy).  Output DMAs alternate between
the SWDGE ring (gpsimd) and the second HWDGE ring (scalar) so the
output stream is not capped by one ring's ~280 GB/s packet rate;
input stays on the sync HWDGE ring, triggered before the w/bias
const DMAs so the stream starts ~1.3us earlier.  I/O is fp16 to
halve HBM traffic (~3e-4 rel err).  Shapes hardcoded from the spec.
"""

import os

import numpy as np

from concourse import bacc, bass, mybir, tile
from concourse.bass_utils import run_bass_kernel_spmd

B, C, L, K = 32, 64, 16384, 3
LOUT = L - K + 1  # 16382
NCORES = 8
BPC = B // NCORES  # 4 batches per core
P = 128  # partitions (2 phases x C)
LH = L // 2  # 8192 phase-cols per batch
MOUT = LOUT // 2  # 8191 output phase-cols per batch
NJ = 512  # PSUM inner chunk (one fp32 bank)

F32 = mybir.dt.float32
F16 = mybir.dt.float16

CH = int(os.environ.get("CONV_CH", "4096"))
BUFS = int(os.environ.get("CONV_BUFS", "6"))
WARMUP = int(os.environ.get("CONV_WARMUP", "8"))

_NC_CACHE = []


def _chunk_lists():
    """Per-batch chunk lists in psum-col units (each sums to MOUT).
    Batch 0 ramps up small so compute starts early; the last batch
    ramps down so the compute-gated tail after the final input is
    short."""
    ramp = [512, 1024, 2048]
    tail = [2048, 1024, 512, 511]
    lists = {}
    for p in range(BPC):
        if p == 0:
            rest = MOUT - sum(ramp)
            body = [CH] * (rest // CH)
            lists[p] = ramp + body + [rest - sum(body)]
        elif p == BPC - 1:
            rest = MOUT - sum(tail)
            body = [CH] * (rest // CH)
            lists[p] = body + [rest - sum(body)] + tail
        else:
            body = [CH] * (MOUT // CH)
            lists[p] = body + [MOUT - sum(body)]
        lists[p] = [n for n in lists[p] if n > 0]
        assert sum(lists[p]) == MOUT, (p, lists[p])
    return lists


def _build_nc():
    nc = bacc.Bacc("TRN2", target_bir_lowering=False, debug=False,
                   num_devices=NCORES)

    x2 = nc.dram_tensor("x2", [BPC, P, LH], F16, kind="ExternalInput")
    wT = nc.dram_tensor("wT", [P, 2, P], F16, kind="ExternalInput")
    b2 = nc.dram_tensor("b2", [P, 1], F32, kind="ExternalInput")
    y2 = nc.dram_tensor("y2", [BPC, P, MOUT], F16, kind="ExternalOutput")

    lists = _chunk_lists()

    with tile.TileContext(nc) as tc:
        with (
            tc.tile_pool(name="const", bufs=1) as const_pool,
            tc.tile_pool(name="inp", bufs=BUFS) as inp_pool,
            tc.tile_pool(name="outp", bufs=BUFS) as outp_pool,
            tc.tile_pool(name="psum", bufs=8, space=bass.MemorySpace.PSUM)
            as psum_pool,
        ):
            # First input chunk's DMA goes on the sync ring before the
            # const DMAs so the input stream starts as early as possible.
            it0 = inp_pool.tile([P, CH + 1], F16, tag="in")
            n0 = lists[0][0]
            nc.sync.dma_start(out=it0[:, :n0 + 1], in_=x2[0, :, :n0 + 1])

            w = const_pool.tile([P, 2, P], F16)
            nc.sync.dma_start(out=w[:], in_=wT[:])
            bias = const_pool.tile([P, 1], F32)
            nc.sync.dma_start(out=bias[:], in_=b2[:])

            # HAM warm-up: dummy matmuls on zeroed SBUF while the first
            # input DMA is in flight, so the PE clock gate is at 8/8
            # (2.4 GHz) when real work arrives.
            if WARMUP:
                wz = const_pool.tile([P, NJ], F16)
                nc.vector.memset(wz[:], 0.0)
                for i in range(WARMUP):
                    wp = psum_pool.tile([P, NJ], F32, tag="acc",
                                        name=f"warm{i}")
                    nc.tensor.matmul(wp[:], wz[:, :P], wz[:],
                                     start=True, stop=True)

            ci = 0  # global psum-chunk counter (ACT/DVE alternation)
            oi = 0  # global output-chunk counter (DMA ring alternation)
            for p in range(BPC):
                m0 = 0
                for li, n in enumerate(lists[p]):
                    if p == 0 and li == 0:
                        it = it0
                    else:
                        it = inp_pool.tile([P, CH + 1], F16, tag="in")
                        nc.sync.dma_start(out=it[:, :n + 1],
                                          in_=x2[p, :, m0:m0 + n + 1])
                    ot = outp_pool.tile([P, CH], F16, tag="out")
                    for j0 in range(0, n, NJ):
                        nj = min(NJ, n - j0)
                        pt = psum_pool.tile([P, NJ], F32, tag="acc")
                        nc.tensor.matmul(pt[:, :nj], w[:, 0, :],
                                         it[:, j0:j0 + nj],
                                         start=True, stop=False)
                        nc.tensor.matmul(pt[:, :nj], w[:, 1, :],
                                         it[:, j0 + 1:j0 + 1 + nj],
                                         start=False, stop=True)
                        # psum -> sbuf with fused bias add; whole chunk
                        # on one engine, alternating ACT/DVE
                        if ci % 2 == 0:
                            nc.scalar.add(ot[:, j0:j0 + nj], pt[:, :nj],
                                          add=bias[:, 0:1])
                        else:
                            nc.vector.tensor_scalar_add(ot[:, j0:j0 + nj],
                                                        pt[:, :nj],
                                                        bias[:, 0:1])
                        ci += 1
                    if oi % 2 == 0:
                        nc.gpsimd.dma_start(out=y2[p, :, m0:m0 + n],
                                            in_=ot[:, :n])
                    else:
                        nc.scalar.dma_start(out=y2[p, :, m0:m0 + n],
                                            in_=ot[:, :n])
                    oi += 1
                    m0 += n

    nc.compile()
    return nc


def _get_nc():
    if not _NC_CACHE:
        _NC_CACHE.append(_build_nc())
    return _NC_CACHE[0]


def _prep_weights(weight, bias):
    w = weight.astype(np.float32)
    wT = np.zeros((P, 2, P), np.float32)
    w0, w1, w2 = w[:, :, 0].T, w[:, :, 1].T, w[:, :, 2].T  # [C_in, C_out]
    wT[0:C, 0, 0:C] = w0
    wT[C:P, 0, 0:C] = w1
    wT[C:P, 0, C:P] = w0
    wT[0:C, 1, 0:C] = w2
    wT[0:C, 1, C:P] = w1
    wT[C:P, 1, C:P] = w2
    b2 = np.concatenate([bias, bias]).reshape(P, 1).astype(np.float32)
    return wT.astype(np.float16), b2


def kernel(x, weight, bias, _want_results=False, **run_kwargs):
    x = np.asarray(x, np.float32)
    weight = np.asarray(weight, np.float32)
    bias = np.asarray(bias, np.float32)
    nc = _get_nc()
    wT, b2 = _prep_weights(weight, bias)

    # de-interleave length into even/odd phases stacked on partitions
    xh = x.astype(np.float16)
    in_maps = []
    for i in range(NCORES):
        xs = xh[BPC * i:BPC * (i + 1)]  # [BPC, C, L]
        xde = np.empty((BPC, P, LH), np.float16)
        xde[:, :C, :] = xs[:, :, 0::2]
        xde[:, C:, :] = xs[:, :, 1::2]
        in_maps.append({"x2": xde, "wT": wT, "b2": b2})

    res = run_bass_kernel_spmd(nc, in_maps, list(range(NCORES)), **run_kwargs)

    out = np.empty((B, C, LOUT), np.float32)
    for i in range(NCORES):
        yde = res.results[i]["y2"]  # [BPC, P, MOUT] f16
        ob = out[BPC * i:BPC * (i + 1)]
        ob[:, :, 0::2] = yde[:, :C, :]
        ob[:, :, 1::2] = yde[:, C:, :]
    if _want_results:
        return out, res
    return out
